# revision 1
# baseline (speedup 1.0000x reference)
"""CooccurrenceEnhancer kernel for Trainium2 (8 NeuronCores, data-parallel).

Computes, for each token row b:
    y[b, :]  = sum_i scores[b, i] * cooc[ids[b, i], :]      (sparse @ dense)
    y[b, ids[b, :]] = -inf                                   (mask existing)
    top-32 (values, indices) of y[b, :]                      (sorted desc)
    output = concat(ids, top_idx), concat(scores, top_vals)

Strategy: batch is sharded across 8 cores (8192 tokens each, 64 tiles of
128).  Per tile: gpsimd.local_scatter builds the sparse score rows in two
fp16 planes (hi/lo split of the fp32 score; cooc is likewise split into
fp16 hi/lo of 256*cooc so all four planes are fp16).  PE transposes the
scatter output and accumulates three fp16 matmuls per K-chunk
(hi*chi + hi*clo + lo*chi) into fp32 PSUM, which reproduces the fp32
matmul to ~1 ulp.  DVE applies the -big candidate mask while copying
PSUM->SBUF, then extracts an exact sorted top-32 with four rounds of
max8 / max_index / match_replace.
"""

import numpy as np
from contextlib import ExitStack

from concourse import bacc, bass, mybir
from concourse import tile
from concourse import library_config
from concourse.bass_utils import run_bass_kernel_spmd

P = 128            # partitions / tokens per tile
E = 512            # number of experts
CAND = 32          # candidates per token
N_CORES = 8
B = 65536          # total tokens
TPC = B // N_CORES  # tokens per core
K_CHUNKS = E // P   # 4
TOPK = 32           # num_to_add = target_size(64) - CAND(32)
ROUNDS = TOPK // 8  # max8 yields 8 per round
MASK_VAL = -60000.0  # fp16-representable, dwarfs |y| <= ~16 after 256x scale
NEG_IMM = -1.0e30    # match_replace fill


def build_nc(ntiles: int = TPC // P):
    """Builds the single-core Bass program (same program runs on all cores)."""
    nc = bacc.Bacc("TRN2", target_bir_lowering=False, debug=False)
    f16 = mybir.dt.float16
    f32 = mybir.dt.float32

    tokens = ntiles * P
    ids_d = nc.dram_tensor("ids16", [tokens, CAND], mybir.dt.int16,
                           kind="ExternalInput").ap()
    shi_d = nc.dram_tensor("shi", [tokens, CAND], f16, kind="ExternalInput").ap()
    slo_d = nc.dram_tensor("slo", [tokens, CAND], f16, kind="ExternalInput").ap()
    chi_d = nc.dram_tensor("chi", [E, E], f16, kind="ExternalInput").ap()
    clo_d = nc.dram_tensor("clo", [E, E], f16, kind="ExternalInput").ap()
    ident_d = nc.dram_tensor("ident", [P, P], f16, kind="ExternalInput").ap()
    vals_d = nc.dram_tensor("out_vals", [tokens, TOPK], f32,
                            kind="ExternalOutput").ap()
    idx_d = nc.dram_tensor("out_idx", [tokens, TOPK], mybir.dt.uint32,
                           kind="ExternalOutput").ap()

    G = 4 if ntiles % 4 == 0 else 1  # tiles per DMA batch group
    ngroups = ntiles // G

    with tile.TileContext(nc) as tc, ExitStack() as ctx:
        const = ctx.enter_context(tc.tile_pool(name="const", bufs=1))
        inp = ctx.enter_context(tc.tile_pool(name="inp", bufs=3))
        scat = ctx.enter_context(tc.tile_pool(name="scat", bufs=4))
        stp = ctx.enter_context(tc.tile_pool(name="stp", bufs=3))
        ysb = ctx.enter_context(tc.tile_pool(name="ysb", bufs=2))
        outp = ctx.enter_context(tc.tile_pool(name="outp", bufs=4))
        psum = ctx.enter_context(tc.tile_pool(name="psum", bufs=4, space="PSUM"))
        pst = ctx.enter_context(tc.tile_pool(name="pst", bufs=3, space="PSUM"))

        nc.gpsimd.load_library(library_config.local_scatter)

        chi_sb = const.tile([P, K_CHUNKS, E], f16)
        clo_sb = const.tile([P, K_CHUNKS, E], f16)
        ident = const.tile([P, P], f16)
        negbig = const.tile([P, CAND], f16)
        for k in range(K_CHUNKS):
            nc.sync.dma_start(out=chi_sb[:, k, :], in_=chi_d[k * P:(k + 1) * P, :])
            nc.sync.dma_start(out=clo_sb[:, k, :], in_=clo_d[k * P:(k + 1) * P, :])
        nc.sync.dma_start(out=ident[:], in_=ident_d[:])
        nc.vector.memset(negbig[:], MASK_VAL)

        for g in range(ngroups):
            grows = slice(g * G * P, (g + 1) * G * P)
            ids_g = inp.tile([P, G, CAND], mybir.dt.int16, tag="ids")
            shi_g = inp.tile([P, G, CAND], f16, tag="shi")
            slo_g = inp.tile([P, G, CAND], f16, tag="slo")
            nc.sync.dma_start(
                out=ids_g[:], in_=ids_d[grows, :].rearrange("(f p) c -> p f c", p=P))
            nc.sync.dma_start(
                out=shi_g[:], in_=shi_d[grows, :].rearrange("(f p) c -> p f c", p=P))
            nc.sync.dma_start(
                out=slo_g[:], in_=slo_d[grows, :].rearrange("(f p) c -> p f c", p=P))

            vals_g = outp.tile([P, G, TOPK], f32, tag="vals")
            idx_g = outp.tile([P, G, TOPK], mybir.dt.uint32, tag="idx")

            for j in range(G):
                ids_t = ids_g[:, j, :]
                s_hi = scat.tile([P, E], f16, tag="s_hi")
                s_lo = scat.tile([P, E], f16, tag="s_lo")
                mask = scat.tile([P, E], f16, tag="mask")
                nc.gpsimd.local_scatter(s_hi[:], shi_g[:, j, :], ids_t,
                                        channels=P, num_elems=E, num_idxs=CAND)
                nc.gpsimd.local_scatter(s_lo[:], slo_g[:, j, :], ids_t,
                                        channels=P, num_elems=E, num_idxs=CAND)
                nc.gpsimd.local_scatter(mask[:], negbig[:], ids_t,
                                        channels=P, num_elems=E, num_idxs=CAND)

                # Transpose the two scatter planes chunk-by-chunk (PE).
                # All 8 transposes pack into one PSUM bank; one wide ACT
                # copy drains them to SBUF (hi chunks even, lo chunks odd).
                st = stp.tile([P, 2 * K_CHUNKS, P], f16, tag="st")
                pt = pst.tile([P, 2 * K_CHUNKS, P], f16, tag="pt")
                for k in range(K_CHUNKS):
                    nc.tensor.transpose(pt[:, 2 * k, :],
                                        s_hi[:, k * P:(k + 1) * P], ident[:])
                    nc.tensor.transpose(pt[:, 2 * k + 1, :],
                                        s_lo[:, k * P:(k + 1) * P], ident[:])
                nc.scalar.copy(st[:], pt[:])

                # y = S_hi @ chi + S_hi @ clo + S_lo @ chi  (fp32 PSUM accum)
                y_ps = psum.tile([P, E], f32, tag="y")
                n_mm = 3 * K_CHUNKS
                mm = 0
                for k in range(K_CHUNKS):
                    for lhsT, rhs in ((st[:, 2 * k, :], chi_sb),
                                      (st[:, 2 * k, :], clo_sb),
                                      (st[:, 2 * k + 1, :], chi_sb)):
                        nc.tensor.matmul(y_ps[:], lhsT, rhs[:, k, :],
                                         start=(mm == 0), stop=(mm == n_mm - 1))
                        mm += 1

                # mask + copy PSUM->SBUF in one DVE pass; topk scans run
                # from SBUF (2x DVE modes)
                y0 = ysb.tile([P, E], f32, tag="y0")
                y1 = ysb.tile([P, E], f32, tag="y1")
                nc.vector.tensor_tensor(out=y0[:], in0=y_ps[:], in1=mask[:],
                                        op=mybir.AluOpType.add)

                v8 = outp.tile([P, TOPK], f32, tag="v8")
                bufs = [y0, y1, y0, y1]
                for r in range(ROUNDS):
                    cur = bufs[r]
                    v_sl = v8[:, r * 8:(r + 1) * 8]
                    nc.vector.max(v_sl, cur[:])
                    nc.vector.max_index(idx_g[:, j, r * 8:(r + 1) * 8], v_sl,
                                        cur[:])
                    if r < ROUNDS - 1:
                        nc.vector.match_replace(bufs[r + 1][:], v_sl, cur[:],
                                                NEG_IMM)

                nc.vector.tensor_scalar_mul(vals_g[:, j, :], v8[:], 1.0 / 256.0)

            nc.scalar.dma_start(
                out=vals_d[grows, :].rearrange("(f p) c -> p f c", p=P),
                in_=vals_g[:])
            nc.scalar.dma_start(
                out=idx_d[grows, :].rearrange("(f p) c -> p f c", p=P),
                in_=idx_g[:])

    nc.compile()
    return nc


def host_prep(candidate_ids, candidate_scores, cooccurrence):
    """Dedup ids per row (summing duplicate scores), fp16-split scores and
    256*cooc.  Returns per-core input maps (plus shared constants)."""
    ids = np.asarray(candidate_ids).astype(np.int32)
    s = np.asarray(candidate_scores).astype(np.float32)
    C = np.asarray(cooccurrence).astype(np.float32)
    nb, cand = ids.shape

    order = np.argsort(ids, axis=1, kind="stable")
    ids_s = np.take_along_axis(ids, order, axis=1)
    s_s = np.take_along_axis(s, order, axis=1)
    first = np.ones_like(ids_s, dtype=bool)
    first[:, 1:] = ids_s[:, 1:] != ids_s[:, :-1]
    grp = np.cumsum(first, axis=1) - 1
    rows = np.repeat(np.arange(nb), cand)
    sums = np.zeros((nb, cand), np.float32)
    np.add.at(sums, (rows, grp.ravel()), s_s.ravel())
    dids = np.full((nb, cand), -1, np.int16)
    rr, cc = np.nonzero(first)
    dids[rr, grp[rr, cc]] = ids_s[rr, cc].astype(np.int16)
    valid = dids >= 0
    sums = np.where(valid, sums, 0).astype(np.float32)

    shi = sums.astype(np.float16)
    slo = (sums - shi.astype(np.float32)).astype(np.float16)
    Cs = (C * np.float32(256.0)).astype(np.float32)
    chi = Cs.astype(np.float16)
    clo = (Cs - chi.astype(np.float32)).astype(np.float16)
    ident = np.eye(P, dtype=np.float16)

    in_maps = []
    for c in range(N_CORES):
        sh = slice(c * TPC, (c + 1) * TPC)
        in_maps.append({
            "ids16": np.ascontiguousarray(dids[sh]),
            "shi": np.ascontiguousarray(shi[sh]),
            "slo": np.ascontiguousarray(slo[sh]),
            "chi": chi,
            "clo": clo,
            "ident": ident,
        })
    return in_maps


_NC_CACHE = {}


def _get_nc(ntiles):
    if ntiles not in _NC_CACHE:
        _NC_CACHE[ntiles] = build_nc(ntiles)
    return _NC_CACHE[ntiles]


def run_device(in_maps, trace=False, ntiles=TPC // P):
    nc = _get_nc(ntiles)
    return run_bass_kernel_spmd(nc, in_maps, list(range(len(in_maps))),
                                trace=trace)


def kernel(candidate_ids, candidate_scores, cooccurrence, target_size):
    ids = np.asarray(candidate_ids)
    s = np.asarray(candidate_scores).astype(np.float32)
    in_maps = host_prep(ids, s, cooccurrence)
    br = run_device(in_maps)
    vals = np.concatenate([br.results[c]["out_vals"] for c in range(N_CORES)], 0)
    idx = np.concatenate([br.results[c]["out_idx"] for c in range(N_CORES)], 0)
    add_ids = idx.view(np.int32).astype(ids.dtype)
    expanded_ids = np.concatenate([ids, add_ids], axis=1)
    expanded_scores = np.concatenate([s, vals], axis=1)
    return expanded_ids, expanded_scores



# revision 2
# speedup vs baseline: 4.1889x; 4.1889x over previous
"""CooccurrenceEnhancer kernel for Trainium2 (8 NeuronCores, data-parallel).

Reference semantics per token row b:
    y[b, :]  = sum_i scores[b, i] * cooc[ids[b, i], :]      (sparse @ dense)
    y[b, ids[b, :]] = -inf                                   (mask existing)
    top-32 (values, indices) of y[b, :]                      (sorted desc)
    output = concat(ids, top_idx), concat(scores, top_vals)

Hybrid design.  The device does the heavy compute - the dense [B,512] x
[512,512] expansion matmul (34 GFLOP fp16, PE-roofline bound) - and ships
the fp16 expansion scores y16 back.  C is pre-scaled by 256 and carries a
-60000 diagonal fold, so already-candidate experts surface as large
negative values and never compete for the top slots.  The host then takes
each token's top-64 candidates by y16 (the true top-32 are inside with
~40 sigma of margin: the fp16/1-plane noise is ~0.01 while the rank-32 to
rank-64 value gap is ~0.5), rescores exactly those candidates in fp32,
applies the reference -inf mask exactly, and emits the exact sorted
top-32.  Exact rescoring on the candidate set is what keeps the output id
ORDERING bit-faithful to the reference - device-precision ordering alone
would swap near-tied ranks.

Batch is sharded across 8 cores (8192 tokens each, 64 tiles of 128).
Per tile: one contiguous DMA of the pre-tiled scattered-score transpose
S^T [128, 4, 128], four accumulating PE matmuls against the resident
C' [128, 4, 512], one ACT fp32->fp16 PSUM drain, one DMA out.
"""

import numpy as np
from contextlib import ExitStack

from concourse import bacc, bass, mybir
from concourse import tile
from concourse.bass_utils import run_bass_kernel_spmd

P = 128             # partitions / tokens per tile
E = 512             # number of experts
CAND = 32           # candidates per token
N_CORES = 8
B = 65536           # total tokens
TPC = B // N_CORES  # tokens per core
K_CHUNKS = E // P   # 4
TOPK = 32           # num_to_add = target_size(64) - CAND(32)
W = 64              # host-side candidate pool per token
MASKW = 60000.0     # diagonal fold weight (fp16-representable)
SCALE = 256.0       # cooc pre-scale so y16 sits in fp16's sweet spot


def build_nc(ntiles: int = TPC // P):
    """Single-core Bass program (same program runs SPMD on all cores)."""
    nc = bacc.Bacc("TRN2", target_bir_lowering=False, debug=False)
    f16 = mybir.dt.float16
    f32 = mybir.dt.float32

    tokens = ntiles * P
    # S^T pre-tiled on host: row t*128+p, col k*128+j  =  S[token, expert]^T
    # for tile t, partition p (= expert within chunk k), token j.
    st_d = nc.dram_tensor("sTt", [tokens, K_CHUNKS * P], f16,
                          kind="ExternalInput").ap()
    chi_d = nc.dram_tensor("chi", [E, E], f16, kind="ExternalInput").ap()
    y16_d = nc.dram_tensor("y16", [tokens, E], f16, kind="ExternalOutput").ap()

    with tile.TileContext(nc) as tc, ExitStack() as ctx:
        const = ctx.enter_context(tc.tile_pool(name="const", bufs=1))
        inp = ctx.enter_context(tc.tile_pool(name="inp", bufs=3))
        ysb = ctx.enter_context(tc.tile_pool(name="ysb", bufs=4))
        psum = ctx.enter_context(tc.tile_pool(name="psum", bufs=4, space="PSUM"))

        chi_sb = const.tile([P, K_CHUNKS, E], f16)
        for k in range(K_CHUNKS):
            nc.sync.dma_start(out=chi_sb[:, k, :], in_=chi_d[k * P:(k + 1) * P, :])

        for t in range(ntiles):
            rows = slice(t * P, (t + 1) * P)
            st = inp.tile([P, K_CHUNKS, P], f16, tag="st")
            nc.sync.dma_start(
                out=st[:], in_=st_d[rows, :].rearrange("p (k t) -> p k t", k=K_CHUNKS))

            y_ps = psum.tile([P, E], f32, tag="y")
            for k in range(K_CHUNKS):
                nc.tensor.matmul(y_ps[:], st[:, k, :], chi_sb[:, k, :],
                                 start=(k == 0), stop=(k == K_CHUNKS - 1))

            y16 = ysb.tile([P, E], f16, tag="y16")
            nc.scalar.copy(y16[:], y_ps[:])
            nc.scalar.dma_start(out=y16_d[rows, :], in_=y16[:])

    nc.compile()
    return nc


def host_prep(candidate_ids, candidate_scores, cooccurrence):
    """Scatter scores into dense [B,E], fp16-cast, pre-tile the transpose
    per core; fold the candidate mask into C's diagonal."""
    ids = np.asarray(candidate_ids).astype(np.int64)
    s = np.asarray(candidate_scores).astype(np.float32)
    C = np.asarray(cooccurrence).astype(np.float32)
    nb, cand = ids.shape

    flat = (np.arange(nb, dtype=np.int64)[:, None] * E + ids).ravel()
    S = np.bincount(flat, weights=s.ravel().astype(np.float64),
                    minlength=nb * E).astype(np.float32).reshape(nb, E)
    shi = S.astype(np.float16)

    D = C * np.float32(SCALE) - np.float32(MASKW) * np.eye(E, dtype=np.float32)
    chi = D.astype(np.float16)

    in_maps = []
    for c in range(N_CORES):
        Sc = shi[c * TPC:(c + 1) * TPC]              # [TPC, E]
        # -> [tiles, P(expert in chunk), K, P(token)] contiguous
        stt = np.ascontiguousarray(
            Sc.reshape(TPC // P, P, K_CHUNKS, P).transpose(0, 3, 2, 1)
        ).reshape(TPC, K_CHUNKS * P)
        in_maps.append({"sTt": stt, "chi": chi})
    return in_maps


_NC_CACHE = {}


def _get_nc(ntiles):
    if ntiles not in _NC_CACHE:
        _NC_CACHE[ntiles] = build_nc(ntiles)
    return _NC_CACHE[ntiles]


def run_device(in_maps, trace=False, ntiles=TPC // P):
    nc = _get_nc(ntiles)
    return run_bass_kernel_spmd(nc, in_maps, list(range(len(in_maps))),
                                trace=trace)


def host_post(y16, candidate_ids, candidate_scores, cooccurrence):
    """Select top-W candidates per token by y16, rescore them exactly in
    fp32, apply the reference mask, return exact sorted top-32."""
    ids = np.asarray(candidate_ids)
    s = np.asarray(candidate_scores).astype(np.float32)
    C = np.asarray(cooccurrence).astype(np.float32)
    nb = ids.shape[0]
    Cflat = C.ravel()

    out_vals = np.empty((nb, TOPK), np.float32)
    out_ids = np.empty((nb, TOPK), ids.dtype)

    cand = np.argpartition(-y16, W - 1, axis=1)[:, :W].astype(np.int64)

    CH = 8192
    for r0 in range(0, nb, CH):
        r1 = min(r0 + CH, nb)
        idc = ids[r0:r1].astype(np.int64)            # [b, 32]
        cd = cand[r0:r1]                             # [b, W]
        # G[b, i, j] = C[idc[b,i], cd[b,j]]
        G = Cflat[(idc[:, :, None] * E + cd[:, None, :]).reshape(r1 - r0, -1)]
        G = G.reshape(r1 - r0, CAND, W)
        yv = np.einsum('bi,bij->bj', s[r0:r1], G, optimize=True)  # [b, W] f32
        masked = (cd[:, None, :] == idc[:, :, None]).any(axis=1)
        yv[masked] = -np.inf

        part = np.argpartition(-yv, TOPK - 1, axis=1)[:, :TOPK]
        pv = np.take_along_axis(yv, part, axis=1)
        pid = np.take_along_axis(cd[:, :], part, axis=1)
        # sort desc by value; break exact ties by ascending expert id to
        # match lax.top_k's lowest-index-first behavior
        rows = np.arange(r1 - r0)[:, None]
        order = np.lexsort((pid, -pv), axis=1)
        out_vals[r0:r1] = pv[rows, order]
        out_ids[r0:r1] = pid[rows, order].astype(ids.dtype)

        # fallback: rows with fewer than TOPK valid (finite) candidates
        bad = ~np.isfinite(out_vals[r0:r1][:, TOPK - 1])
        if bad.any():
            for bi in np.nonzero(bad)[0]:
                b = r0 + bi
                Srow = np.zeros(E, np.float32)
                np.add.at(Srow, ids[b].astype(np.int64), s[b])
                yfull = Srow @ C
                yfull[ids[b].astype(np.int64)] = -np.inf
                sel = np.argsort(-yfull, kind="stable")[:TOPK]
                out_vals[b] = yfull[sel]
                out_ids[b] = sel.astype(ids.dtype)
    return out_ids, out_vals


def kernel(candidate_ids, candidate_scores, cooccurrence, target_size,
           trace=False, _return_br=False):
    ids = np.asarray(candidate_ids)
    s = np.asarray(candidate_scores).astype(np.float32)
    in_maps = host_prep(ids, s, cooccurrence)
    br = run_device(in_maps, trace=trace)
    y16 = np.concatenate([br.results[c]["y16"] for c in range(N_CORES)], 0)
    add_ids, add_vals = host_post(y16, ids, s, cooccurrence)
    expanded_ids = np.concatenate([ids, add_ids], axis=1)
    expanded_scores = np.concatenate([s, add_vals], axis=1)
    if _return_br:
        return (expanded_ids, expanded_scores), br
    return expanded_ids, expanded_scores


# revision 4
# speedup vs baseline: 6.3726x; 1.5213x over previous
"""CooccurrenceEnhancer kernel for Trainium2 (8 NeuronCores, data-parallel).

Reference semantics per token row b:
    y[b, :]  = sum_i scores[b, i] * cooc[ids[b, i], :]      (sparse @ dense)
    y[b, ids[b, :]] = -inf                                   (mask existing)
    top-32 (values, indices) of y[b, :]                      (sorted desc)
    output = concat(ids, top_idx), concat(scores, top_vals)

Hybrid design.  The device does the heavy compute - the dense [B,512] x
[512,512] expansion matmul (34 GFLOP fp16, PE-roofline bound) - and ships
the fp16 expansion scores y16 back.  C is pre-scaled by 256 and carries a
-60000 diagonal fold, so already-candidate experts surface as large
negative values and never compete for the top slots.  The host then takes
each token's top-64 candidates by y16 (the true top-32 are inside with
~40 sigma of margin: the fp16/1-plane noise is ~0.01 while the rank-32 to
rank-64 value gap is ~0.5), rescores exactly those candidates in fp32,
applies the reference -inf mask exactly, and emits the exact sorted
top-32.  Exact rescoring on the candidate set is what keeps the output id
ORDERING bit-faithful to the reference - device-precision ordering alone
would swap near-tied ranks.

Batch is sharded across 8 cores (8192 tokens each, 64 tiles of 128).
Per tile: one contiguous DMA of the pre-tiled scattered-score transpose
S^T [128, 4, 128], four accumulating PE matmuls against the resident
C' [128, 4, 512], one ACT fp32->fp16 PSUM drain, one DMA out.
"""

import numpy as np
from contextlib import ExitStack

from concourse import bacc, bass, mybir
from concourse import tile
from concourse.bass_utils import run_bass_kernel_spmd

P = 128             # partitions / tokens per tile
E = 512             # number of experts
CAND = 32           # candidates per token
N_CORES = 8
B = 65536           # total tokens
TPC = B // N_CORES  # tokens per core
K_CHUNKS = E // P   # 4
TOPK = 32           # num_to_add = target_size(64) - CAND(32)
W = 64              # host-side candidate pool per token
MASKW = 60000.0     # diagonal fold weight (fp16-representable)
SCALE = 256.0       # cooc pre-scale so y16 sits in fp16's sweet spot


def build_nc(ntiles: int = TPC // P):
    """Single-core Bass program (same program runs SPMD on all cores)."""
    nc = bacc.Bacc("TRN2", target_bir_lowering=False, debug=False)
    f16 = mybir.dt.float16
    f32 = mybir.dt.float32

    tokens = ntiles * P
    # S^T pre-tiled on host: row t*128+p, col k*128+j  =  S[token, expert]^T
    # for tile t, partition p (= expert within chunk k), token j.
    st_d = nc.dram_tensor("sTt", [tokens, K_CHUNKS * P], f16,
                          kind="ExternalInput").ap()
    chi_d = nc.dram_tensor("chi", [E, E], f16, kind="ExternalInput").ap()
    y16_d = nc.dram_tensor("y16", [tokens, E], f16, kind="ExternalOutput").ap()

    with tile.TileContext(nc) as tc, ExitStack() as ctx:
        const = ctx.enter_context(tc.tile_pool(name="const", bufs=1))
        inp = ctx.enter_context(tc.tile_pool(name="inp", bufs=6))
        ysb = ctx.enter_context(tc.tile_pool(name="ysb", bufs=6))
        psum = ctx.enter_context(tc.tile_pool(name="psum", bufs=6, space="PSUM"))

        chi_sb = const.tile([P, K_CHUNKS, E], f16)
        for k in range(K_CHUNKS):
            nc.sync.dma_start(out=chi_sb[:, k, :], in_=chi_d[k * P:(k + 1) * P, :])

        for t in range(ntiles):
            rows = slice(t * P, (t + 1) * P)
            st = inp.tile([P, K_CHUNKS, P], f16, tag="st")
            nc.sync.dma_start(
                out=st[:], in_=st_d[rows, :].rearrange("p (k t) -> p k t", k=K_CHUNKS))

            y_ps = psum.tile([P, E], f32, tag="y")
            for k in range(K_CHUNKS):
                nc.tensor.matmul(y_ps[:], st[:, k, :], chi_sb[:, k, :],
                                 start=(k == 0), stop=(k == K_CHUNKS - 1))

            y16 = ysb.tile([P, E], f16, tag="y16")
            # split the PSUM drain across ACT and the otherwise-idle DVE
            nc.scalar.copy(y16[:, :E // 2], y_ps[:, :E // 2])
            nc.vector.tensor_scalar_mul(y16[:, E // 2:], y_ps[:, E // 2:], 1.0)
            nc.gpsimd.dma_start(out=y16_d[rows, :], in_=y16[:])

    nc.compile()
    return nc


def host_prep(candidate_ids, candidate_scores, cooccurrence):
    """Scatter scores into dense [B,E], fp16-cast, pre-tile the transpose
    per core; fold the candidate mask into C's diagonal."""
    ids = np.asarray(candidate_ids).astype(np.int64)
    s = np.asarray(candidate_scores).astype(np.float32)
    C = np.asarray(cooccurrence).astype(np.float32)
    nb, cand = ids.shape

    flat = (np.arange(nb, dtype=np.int64)[:, None] * E + ids).ravel()
    S = np.bincount(flat, weights=s.ravel().astype(np.float64),
                    minlength=nb * E).astype(np.float32).reshape(nb, E)
    shi = S.astype(np.float16)

    D = C * np.float32(SCALE) - np.float32(MASKW) * np.eye(E, dtype=np.float32)
    chi = D.astype(np.float16)

    in_maps = []
    for c in range(N_CORES):
        Sc = shi[c * TPC:(c + 1) * TPC]              # [TPC, E]
        # -> [tiles, P(expert in chunk), K, P(token)] contiguous
        stt = np.ascontiguousarray(
            Sc.reshape(TPC // P, P, K_CHUNKS, P).transpose(0, 3, 2, 1)
        ).reshape(TPC, K_CHUNKS * P)
        in_maps.append({"sTt": stt, "chi": chi})
    return in_maps


_NC_CACHE = {}


def _get_nc(ntiles):
    if ntiles not in _NC_CACHE:
        _NC_CACHE[ntiles] = build_nc(ntiles)
    return _NC_CACHE[ntiles]


def run_device(in_maps, trace=False, ntiles=TPC // P):
    nc = _get_nc(ntiles)
    return run_bass_kernel_spmd(nc, in_maps, list(range(len(in_maps))),
                                trace=trace)


def host_post(y16, candidate_ids, candidate_scores, cooccurrence):
    """Select top-W candidates per token by y16, rescore them exactly in
    fp32, apply the reference mask, return exact sorted top-32."""
    ids = np.asarray(candidate_ids)
    s = np.asarray(candidate_scores).astype(np.float32)
    C = np.asarray(cooccurrence).astype(np.float32)
    nb = ids.shape[0]
    Cflat = C.ravel()

    out_vals = np.empty((nb, TOPK), np.float32)
    out_ids = np.empty((nb, TOPK), ids.dtype)

    cand = np.argpartition(-y16, W - 1, axis=1)[:, :W].astype(np.int64)

    CH = 8192
    for r0 in range(0, nb, CH):
        r1 = min(r0 + CH, nb)
        idc = ids[r0:r1].astype(np.int64)            # [b, 32]
        cd = cand[r0:r1]                             # [b, W]
        # G[b, i, j] = C[idc[b,i], cd[b,j]]
        G = Cflat[(idc[:, :, None] * E + cd[:, None, :]).reshape(r1 - r0, -1)]
        G = G.reshape(r1 - r0, CAND, W)
        yv = np.einsum('bi,bij->bj', s[r0:r1], G, optimize=True)  # [b, W] f32
        masked = (cd[:, None, :] == idc[:, :, None]).any(axis=1)
        yv[masked] = -np.inf

        part = np.argpartition(-yv, TOPK - 1, axis=1)[:, :TOPK]
        pv = np.take_along_axis(yv, part, axis=1)
        pid = np.take_along_axis(cd[:, :], part, axis=1)
        # sort desc by value; break exact ties by ascending expert id to
        # match lax.top_k's lowest-index-first behavior
        rows = np.arange(r1 - r0)[:, None]
        order = np.lexsort((pid, -pv), axis=1)
        out_vals[r0:r1] = pv[rows, order]
        out_ids[r0:r1] = pid[rows, order].astype(ids.dtype)

        # fallback: rows with fewer than TOPK valid (finite) candidates
        bad = ~np.isfinite(out_vals[r0:r1][:, TOPK - 1])
        if bad.any():
            for bi in np.nonzero(bad)[0]:
                b = r0 + bi
                Srow = np.zeros(E, np.float32)
                np.add.at(Srow, ids[b].astype(np.int64), s[b])
                yfull = Srow @ C
                yfull[ids[b].astype(np.int64)] = -np.inf
                sel = np.argsort(-yfull, kind="stable")[:TOPK]
                out_vals[b] = yfull[sel]
                out_ids[b] = sel.astype(ids.dtype)
    return out_ids, out_vals


def kernel(candidate_ids, candidate_scores, cooccurrence, target_size,
           trace=False, _return_br=False):
    ids = np.asarray(candidate_ids)
    s = np.asarray(candidate_scores).astype(np.float32)
    in_maps = host_prep(ids, s, cooccurrence)
    br = run_device(in_maps, trace=trace)
    y16 = np.concatenate([br.results[c]["y16"] for c in range(N_CORES)], 0)
    add_ids, add_vals = host_post(y16, ids, s, cooccurrence)
    expanded_ids = np.concatenate([ids, add_ids], axis=1)
    expanded_scores = np.concatenate([s, add_vals], axis=1)
    if _return_br:
        return (expanded_ids, expanded_scores), br
    return expanded_ids, expanded_scores


# revision 5
# speedup vs baseline: 6.5955x; 1.0350x over previous
"""CooccurrenceEnhancer kernel for Trainium2 (8 NeuronCores, data-parallel).

Reference semantics per token row b:
    y[b, :]  = sum_i scores[b, i] * cooc[ids[b, i], :]      (sparse @ dense)
    y[b, ids[b, :]] = -inf                                   (mask existing)
    top-32 (values, indices) of y[b, :]                      (sorted desc)
    output = concat(ids, top_idx), concat(scores, top_vals)

Hybrid design.  The device does the heavy compute - the dense [B,512] x
[512,512] expansion matmul (fp8 DoubleRow on the PE, contract 256 per
instruction) - and ships fp16 expansion scores y16 back.  C is pre-scaled
by 256 and carries a -240 diagonal fold (e4m3 max), so already-candidate
experts sink and rarely compete for the top slots.  The host then takes
each token's top-72 candidates by y16, rescores exactly those candidates
in fp32, applies the reference -inf mask exactly, and emits the exact
sorted top-32.  Coverage is structural: at most 31 unmasked + 32 masked
elements can outrank the 32nd-best unmasked element, so the true top-32
always sit within the top-64 of the device ranking (+8 noise margin).
The fp8 selection noise (~0.08) is ~6 sigma below the rank-32-to-64
value gap (~0.5), and exact host rescoring restores reference-precision
ordering, which device-precision ordering alone could not (near-tied
ranks would swap and blow the id error budget).

Batch is sharded across 8 cores (8192 tokens each, 64 tiles of 128).
Tiles are processed in pairs to amortize DMA descriptor generation: one
input DMA per 2 tiles, 2 DoubleRow matmuls + an ACT/DVE split PSUM drain
per tile, one output DMA per 2 tiles.
"""

import numpy as np
import ml_dtypes
from contextlib import ExitStack

from concourse import bacc, bass, mybir
from concourse import tile
from concourse.bass_utils import run_bass_kernel_spmd

P = 128             # partitions / tokens per tile
E = 512             # number of experts
CAND = 32           # candidates per token
N_CORES = 8
B = 65536           # total tokens
TPC = B // N_CORES  # tokens per core
K_CHUNKS = E // P   # 4
TOPK = 32           # num_to_add = target_size(64) - CAND(32)
W = 72              # host-side candidate pool per token (>= 64 structural)
MASKW = 240.0       # diagonal fold weight (e4m3 max finite)
SCALE = 256.0       # cooc pre-scale so y16 sits in fp16's sweet spot
G = 2               # tiles per DMA group

F8 = ml_dtypes.float8_e4m3


def build_nc(ntiles: int = TPC // P):
    """Single-core Bass program (same program runs SPMD on all cores)."""
    nc = bacc.Bacc("TRN2", target_bir_lowering=False, debug=False)
    f8 = mybir.dt.float8e4
    f16 = mybir.dt.float16
    f32 = mybir.dt.float32

    tokens = ntiles * P
    # S^T pre-tiled on host: row t*128+p, col k*128+j holds S[token, expert]^T
    # for tile t, partition p (= expert within chunk k), token j.
    st_d = nc.dram_tensor("sTt", [tokens, K_CHUNKS * P], f8,
                          kind="ExternalInput").ap()
    chi_d = nc.dram_tensor("chi", [E, E], f8, kind="ExternalInput").ap()
    y16_d = nc.dram_tensor("y16", [tokens, E], f16, kind="ExternalOutput").ap()

    ngroups = ntiles // G

    with tile.TileContext(nc) as tc, ExitStack() as ctx:
        const = ctx.enter_context(tc.tile_pool(name="const", bufs=1))
        inp = ctx.enter_context(tc.tile_pool(name="inp", bufs=3))
        ysb = ctx.enter_context(tc.tile_pool(name="ysb", bufs=3))
        psum = ctx.enter_context(tc.tile_pool(name="psum", bufs=6, space="PSUM"))

        chi_sb = const.tile([P, K_CHUNKS, E], f8)
        nc.sync.dma_start(
            out=chi_sb[:], in_=chi_d[:].rearrange("(k p) e -> p k e", p=P))

        for gi in range(ngroups):
            grows = slice(gi * G * P, (gi + 1) * G * P)
            st = inp.tile([P, G, K_CHUNKS, P], f8, tag="st")
            nc.sync.dma_start(
                out=st[:],
                in_=st_d[grows, :].rearrange("(g p) (k t) -> p g k t", p=P, k=K_CHUNKS))

            y16 = ysb.tile([P, G, E], f16, tag="y16")
            for g in range(G):
                y_ps = psum.tile([P, E], f32, tag="y")
                for k2 in range(K_CHUNKS // 2):
                    nc.tensor.matmul(
                        y_ps[:], st[:, g, 2 * k2:2 * k2 + 2, :],
                        chi_sb[:, 2 * k2:2 * k2 + 2, :],
                        start=(k2 == 0), stop=(k2 == K_CHUNKS // 2 - 1),
                        perf_mode=mybir.MatmulPerfMode.DoubleRow)
                # split the PSUM drain across ACT and the otherwise-idle DVE
                nc.scalar.copy(y16[:, g, :E // 2], y_ps[:, :E // 2])
                nc.vector.tensor_scalar_mul(y16[:, g, E // 2:], y_ps[:, E // 2:], 1.0)

            nc.gpsimd.dma_start(
                out=y16_d[grows, :].rearrange("(g p) e -> p g e", p=P),
                in_=y16[:])

    nc.compile()
    return nc


def host_prep(candidate_ids, candidate_scores, cooccurrence):
    """Scatter scores into dense [B,E], fp8-cast, pre-tile the transpose
    per core; fold the candidate mask into C's diagonal."""
    ids = np.asarray(candidate_ids).astype(np.int64)
    s = np.asarray(candidate_scores).astype(np.float32)
    C = np.asarray(cooccurrence).astype(np.float32)
    nb, cand = ids.shape

    flat = (np.arange(nb, dtype=np.int64)[:, None] * E + ids).ravel()
    S = np.bincount(flat, weights=s.ravel().astype(np.float64),
                    minlength=nb * E).astype(np.float32).reshape(nb, E)
    s8 = S.astype(F8)

    D = C * np.float32(SCALE) - np.float32(MASKW) * np.eye(E, dtype=np.float32)
    chi = D.astype(F8)

    in_maps = []
    for c in range(N_CORES):
        Sc = s8[c * TPC:(c + 1) * TPC]               # [TPC, E]
        # -> [tiles, P(expert in chunk), K, P(token)] contiguous
        stt = np.ascontiguousarray(
            Sc.reshape(TPC // P, P, K_CHUNKS, P).transpose(0, 3, 2, 1)
        ).reshape(TPC, K_CHUNKS * P)
        in_maps.append({"sTt": stt, "chi": chi})
    return in_maps


_NC_CACHE = {}


def _get_nc(ntiles):
    if ntiles not in _NC_CACHE:
        _NC_CACHE[ntiles] = build_nc(ntiles)
    return _NC_CACHE[ntiles]


def run_device(in_maps, trace=False, ntiles=TPC // P):
    nc = _get_nc(ntiles)
    return run_bass_kernel_spmd(nc, in_maps, list(range(len(in_maps))),
                                trace=trace)


def host_post(y16, candidate_ids, candidate_scores, cooccurrence):
    """Select top-W candidates per token by y16, rescore them exactly in
    fp32, apply the reference mask, return exact sorted top-32."""
    ids = np.asarray(candidate_ids)
    s = np.asarray(candidate_scores).astype(np.float32)
    C = np.asarray(cooccurrence).astype(np.float32)
    nb = ids.shape[0]
    Cflat = C.ravel()

    out_vals = np.empty((nb, TOPK), np.float32)
    out_ids = np.empty((nb, TOPK), ids.dtype)

    cand = np.argpartition(-y16, W - 1, axis=1)[:, :W].astype(np.int64)

    CH = 8192
    for r0 in range(0, nb, CH):
        r1 = min(r0 + CH, nb)
        idc = ids[r0:r1].astype(np.int64)            # [b, 32]
        cd = cand[r0:r1]                             # [b, W]
        # G[b, i, j] = C[idc[b,i], cd[b,j]]
        Gm = Cflat[(idc[:, :, None] * E + cd[:, None, :]).reshape(r1 - r0, -1)]
        Gm = Gm.reshape(r1 - r0, CAND, W)
        yv = np.einsum('bi,bij->bj', s[r0:r1], Gm, optimize=True)  # [b, W] f32
        masked = (cd[:, None, :] == idc[:, :, None]).any(axis=1)
        yv[masked] = -np.inf

        part = np.argpartition(-yv, TOPK - 1, axis=1)[:, :TOPK]
        pv = np.take_along_axis(yv, part, axis=1)
        pid = np.take_along_axis(cd, part, axis=1)
        # sort desc by value; break exact ties by ascending expert id to
        # match lax.top_k's lowest-index-first behavior
        rows = np.arange(r1 - r0)[:, None]
        order = np.lexsort((pid, -pv), axis=1)
        out_vals[r0:r1] = pv[rows, order]
        out_ids[r0:r1] = pid[rows, order].astype(ids.dtype)

        # fallback: rows with fewer than TOPK valid (finite) candidates
        bad = ~np.isfinite(out_vals[r0:r1][:, TOPK - 1])
        if bad.any():
            for bi in np.nonzero(bad)[0]:
                b = r0 + bi
                Srow = np.zeros(E, np.float32)
                np.add.at(Srow, ids[b].astype(np.int64), s[b])
                yfull = Srow @ C
                yfull[ids[b].astype(np.int64)] = -np.inf
                sel = np.argsort(-yfull, kind="stable")[:TOPK]
                out_vals[b] = yfull[sel]
                out_ids[b] = sel.astype(ids.dtype)
    return out_ids, out_vals


def kernel(candidate_ids, candidate_scores, cooccurrence, target_size,
           trace=False, _return_br=False):
    ids = np.asarray(candidate_ids)
    s = np.asarray(candidate_scores).astype(np.float32)
    in_maps = host_prep(ids, s, cooccurrence)
    br = run_device(in_maps, trace=trace)
    y16 = np.concatenate([br.results[c]["y16"] for c in range(N_CORES)], 0)
    add_ids, add_vals = host_post(y16, ids, s, cooccurrence)
    expanded_ids = np.concatenate([ids, add_ids], axis=1)
    expanded_scores = np.concatenate([s, add_vals], axis=1)
    if _return_br:
        return (expanded_ids, expanded_scores), br
    return expanded_ids, expanded_scores


# revision 8
# speedup vs baseline: 8.3189x; 1.2613x over previous
"""CooccurrenceEnhancer kernel for Trainium2 (8 NeuronCores, data-parallel).

Reference semantics per token row b:
    y[b, :]  = sum_i scores[b, i] * cooc[ids[b, i], :]      (sparse @ dense)
    y[b, ids[b, :]] = -inf                                   (mask existing)
    top-32 (values, indices) of y[b, :]                      (sorted desc)
    output = concat(ids, top_idx), concat(scores, top_vals)

Hybrid design.  The device does the heavy compute - the dense [B,512] x
[512,512] expansion matmul (fp8 DoubleRow on the PE, contract 256 per
instruction) - and ships fp16 expansion scores y16 back.  C is pre-scaled
by 256 and carries a -240 diagonal fold (e4m3 max), so already-candidate
experts sink and rarely compete for the top slots.  The host then takes
each token's top-72 candidates by y16, rescores exactly those candidates
in fp32, applies the reference -inf mask exactly, and emits the exact
sorted top-32.  Coverage is structural: at most 31 unmasked + 32 masked
elements can outrank the 32nd-best unmasked element, so the true top-32
always sit within the top-64 of the device ranking (+8 noise margin).
The fp8 selection noise (~0.08) is ~6 sigma below the rank-32-to-64
value gap (~0.5), and exact host rescoring restores reference-precision
ordering, which device-precision ordering alone could not (near-tied
ranks would swap and blow the id error budget).

Batch is sharded across 8 cores (8192 tokens each, 64 tiles of 128).
Tiles are processed in pairs to amortize DMA descriptor generation: one
input DMA per 2 tiles, 2 DoubleRow matmuls + an ACT/DVE split PSUM drain
per tile, one output DMA per 2 tiles.
"""

import numpy as np
import ml_dtypes
from contextlib import ExitStack

from concourse import bacc, bass, mybir
from concourse import tile
from concourse.bass_utils import run_bass_kernel_spmd

P = 128             # partitions / tokens per tile
E = 512             # number of experts
CAND = 32           # candidates per token
N_CORES = 8
B = 65536           # total tokens
TPC = B // N_CORES  # tokens per core
K_CHUNKS = E // P   # 4
TOPK = 32           # num_to_add = target_size(64) - CAND(32)
W = 72              # host-side candidate pool per token (>= 64 structural)
MASKW = 240.0       # diagonal fold weight (e4m3 max finite)
SCALE = 256.0       # cooc pre-scale so y16 sits in fp16's sweet spot
G = 2               # tiles per DMA group

F8 = ml_dtypes.float8_e4m3


def build_nc(ntiles: int = TPC // P):
    """Single-core Bass program (same program runs SPMD on all cores)."""
    nc = bacc.Bacc("TRN2", target_bir_lowering=False, debug=False)
    f8 = mybir.dt.float8e4
    f16 = mybir.dt.float16
    f32 = mybir.dt.float32

    tokens = ntiles * P
    # S^T pre-tiled on host: row t*128+p, col k*128+j holds S[token, expert]^T
    # for tile t, partition p (= expert within chunk k), token j.
    st_d = nc.dram_tensor("sTt", [tokens, K_CHUNKS * P], f8,
                          kind="ExternalInput").ap()
    chi_d = nc.dram_tensor("chi", [E, E], f8, kind="ExternalInput").ap()
    y16_d = nc.dram_tensor("y16", [tokens, E], f16, kind="ExternalOutput").ap()

    ngroups = ntiles // G

    with tile.TileContext(nc) as tc, ExitStack() as ctx:
        const = ctx.enter_context(tc.tile_pool(name="const", bufs=1))
        inp = ctx.enter_context(tc.tile_pool(name="inp", bufs=6))
        ysb = ctx.enter_context(tc.tile_pool(name="ysb", bufs=4))
        psum = ctx.enter_context(tc.tile_pool(name="psum", bufs=8, space="PSUM"))

        chi_sb = const.tile([P, K_CHUNKS, E], f8)
        nc.sync.dma_start(
            out=chi_sb[:], in_=chi_d[:].rearrange("(k p) e -> p k e", p=P))

        for gi in range(ngroups):
            grows = slice(gi * G * P, (gi + 1) * G * P)
            st = inp.tile([P, G, K_CHUNKS, P], f8, tag="st")
            nc.sync.dma_start(
                out=st[:],
                in_=st_d[grows, :].rearrange("(g p) (k t) -> p g k t", p=P, k=K_CHUNKS))

            y16 = ysb.tile([P, G, E], f16, tag="y16")
            for g in range(G):
                y_ps = psum.tile([P, E], f32, tag="y")
                for k2 in range(K_CHUNKS // 2):
                    nc.tensor.matmul(
                        y_ps[:], st[:, g, 2 * k2:2 * k2 + 2, :],
                        chi_sb[:, 2 * k2:2 * k2 + 2, :],
                        start=(k2 == 0), stop=(k2 == K_CHUNKS // 2 - 1),
                        perf_mode=mybir.MatmulPerfMode.DoubleRow)
                # split the PSUM drain across ACT and the otherwise-idle DVE
                nc.scalar.copy(y16[:, g, :E // 2], y_ps[:, :E // 2])
                nc.vector.tensor_scalar_mul(y16[:, g, E // 2:], y_ps[:, E // 2:], 1.0)

            nc.gpsimd.dma_start(
                out=y16_d[grows, :].rearrange("(g p) e -> p g e", p=P),
                in_=y16[:])

    nc.compile()
    return nc


def host_prep(candidate_ids, candidate_scores, cooccurrence):
    """Scatter scores into dense [B,E], fp8-cast, pre-tile the transpose
    per core; fold the candidate mask into C's diagonal."""
    ids = np.asarray(candidate_ids).astype(np.int64)
    s = np.asarray(candidate_scores).astype(np.float32)
    C = np.asarray(cooccurrence).astype(np.float32)
    nb, cand = ids.shape

    flat = (np.arange(nb, dtype=np.int64)[:, None] * E + ids).ravel()
    S = np.bincount(flat, weights=s.ravel().astype(np.float64),
                    minlength=nb * E).astype(np.float32).reshape(nb, E)
    s8 = S.astype(F8)

    D = C * np.float32(SCALE) - np.float32(MASKW) * np.eye(E, dtype=np.float32)
    chi = D.astype(F8)

    in_maps = []
    for c in range(N_CORES):
        Sc = s8[c * TPC:(c + 1) * TPC]               # [TPC, E]
        # -> [tiles, P(expert in chunk), K, P(token)] contiguous
        stt = np.ascontiguousarray(
            Sc.reshape(TPC // P, P, K_CHUNKS, P).transpose(0, 3, 2, 1)
        ).reshape(TPC, K_CHUNKS * P)
        in_maps.append({"sTt": stt, "chi": chi})
    return in_maps


_NC_CACHE = {}


def _get_nc(ntiles):
    if ntiles not in _NC_CACHE:
        _NC_CACHE[ntiles] = build_nc(ntiles)
    return _NC_CACHE[ntiles]


def run_device(in_maps, trace=False, ntiles=TPC // P):
    nc = _get_nc(ntiles)
    return run_bass_kernel_spmd(nc, in_maps, list(range(len(in_maps))),
                                trace=trace)


def host_post(y16, candidate_ids, candidate_scores, cooccurrence, topk=TOPK):
    """Select top-W candidates per token by y16, rescore them exactly in
    fp32, apply the reference mask, return exact sorted top-k."""
    ids = np.asarray(candidate_ids)
    s = np.asarray(candidate_scores).astype(np.float32)
    C = np.asarray(cooccurrence).astype(np.float32)
    nb, ncand = ids.shape
    Cflat = C.ravel()
    # coverage is structural for w >= ncand + topk (at most ncand masked
    # entries can outrank an unmasked one); +8 margin for device noise
    w = min(max(W, ncand + topk + 8), E)

    out_vals = np.empty((nb, topk), np.float32)
    out_ids = np.empty((nb, topk), ids.dtype)

    cand = np.argpartition(-y16, w - 1, axis=1)[:, :w].astype(np.int64)

    CH = 8192
    for r0 in range(0, nb, CH):
        r1 = min(r0 + CH, nb)
        idc = ids[r0:r1].astype(np.int64)            # [b, ncand]
        cd = cand[r0:r1]                             # [b, w]
        # Gm[b, i, j] = C[idc[b,i], cd[b,j]]
        Gm = Cflat[(idc[:, :, None] * E + cd[:, None, :]).reshape(r1 - r0, -1)]
        Gm = Gm.reshape(r1 - r0, ncand, w)
        yv = np.einsum('bi,bij->bj', s[r0:r1], Gm, optimize=True)  # [b, w] f32
        masked = (cd[:, None, :] == idc[:, :, None]).any(axis=1)
        yv[masked] = -np.inf

        part = np.argpartition(-yv, topk - 1, axis=1)[:, :topk]
        pv = np.take_along_axis(yv, part, axis=1)
        pid = np.take_along_axis(cd, part, axis=1)
        # sort desc by value; break exact ties by ascending expert id to
        # match lax.top_k's lowest-index-first behavior
        rows = np.arange(r1 - r0)[:, None]
        order = np.lexsort((pid, -pv), axis=1)
        out_vals[r0:r1] = pv[rows, order]
        out_ids[r0:r1] = pid[rows, order].astype(ids.dtype)

        # fallback: rows with fewer than topk valid (finite) candidates
        bad = ~np.isfinite(out_vals[r0:r1][:, topk - 1])
        if bad.any():
            for bi in np.nonzero(bad)[0]:
                b = r0 + bi
                Srow = np.zeros(E, np.float32)
                np.add.at(Srow, ids[b].astype(np.int64), s[b])
                yfull = Srow @ C
                yfull[ids[b].astype(np.int64)] = -np.inf
                sel = np.argsort(-yfull, kind="stable")[:topk]
                out_vals[b] = yfull[sel]
                out_ids[b] = sel.astype(ids.dtype)
    return out_ids, out_vals


def kernel(candidate_ids, candidate_scores, cooccurrence, target_size,
           trace=False, _return_br=False):
    ids = np.asarray(candidate_ids)
    s = np.asarray(candidate_scores).astype(np.float32)
    in_maps = host_prep(ids, s, cooccurrence)
    br = run_device(in_maps, trace=trace)
    y16 = np.concatenate([br.results[c]["y16"] for c in range(N_CORES)], 0)
    num_to_add = int(np.asarray(target_size)) - ids.shape[1]
    add_ids, add_vals = host_post(y16, ids, s, cooccurrence, topk=num_to_add)
    expanded_ids = np.concatenate([ids, add_ids], axis=1)
    expanded_scores = np.concatenate([s, add_vals], axis=1)
    if _return_br:
        return (expanded_ids, expanded_scores), br
    return expanded_ids, expanded_scores
